# revision 1
# baseline (speedup 1.0000x reference)
"""Trainium2 Bass kernel for nn_MultiHeadedAttention_64665027608991.

Sparse (per-frame-masked) multi-head attention over B=512 samples, L=176
(8 frames x 22 joints), 8 heads x 64 dims, fp32 I/O.

Strategy: pure data parallel over batch (64 samples per NeuronCore x 8).
Per sample, fully unrolled:
  - x^T (host-pre-transposed) -> q^T/k^T via fp32r matmuls (tf32-class),
    biases folded into the PSUM->SBUF copy on ScalarE.
  - v natural layout with bias via K=1 ones matmul, ReLU on ScalarE into a
    ones-augmented bf16 tile (65 cols per head; col 64 = 1.0 for row sums).
  - scores S^T[k,q] per head via fp32r matmuls; the temporal mask is added
    in PSUM by a constant-matrix matmul (lhsT=I88, rhs=-100 on masked pairs);
    exp on ScalarE (no max subtraction: |scores| <= ~3) -> bf16 P^T.
  - O^T = [v|1]^T @ P^T (bf16): row 64 gives softmax denominators; recip on
    VectorE, broadcast via K=1 matmul, normalize on VectorE.
  - final projection from O^T slices (fp32r) + bias, DMA out.
"""

import sys

sys.path.insert(0, "/opt/trn_rl_repo")

import json

import numpy as np

import concourse.bass as bass
import concourse.tile as tile
from concourse import mybir
from concourse.bass_utils import run_bass_kernel_spmd

DT = mybir.dt

N_CORES = 8
B = 512
BS = B // N_CORES  # 64 samples per core
L = 176
FRAME = 22
NFRAME = 8
IN_DIM = 128
D_MODEL = 512
H_NUM = 8
H_DIM = 64
OUT_DIM = 512
SCALE = 1.0 / np.sqrt(np.float32(H_DIM))
NEG = -100.0  # exp(s + NEG) < 1e-40 for |s| < 10 -> masked weights vanish


# ---------------------------------------------------------------------------
# Workaround: the walrus build in this container rejects instructions with
# more than one sync-wait. Split extras onto single-wait EventSemaphore
# carriers on the same engine.
def _split_multiwaits(bir_json_bytes: bytes) -> bytes:
    j = json.loads(bir_json_bytes)
    n = [0]

    def fix_block(b):
        insts = b.get("instructions")
        if insts:
            out = []
            for inst in insts:
                si = inst.get("sync_info")
                waits = (si or {}).get("on_wait") or []
                if len(waits) > 1:
                    for w in waits[:-1]:
                        n[0] += 1
                        out.append({
                            "name": f"waitfix_{n[0]}",
                            "opcode": "EventSemaphore",
                            "engine": inst.get("engine"),
                            "ins": [],
                            "outs": [],
                            "sync_info": {"on_update": [], "on_wait": [w]},
                        })
                    si["on_wait"] = [waits[-1]]
                out.append(inst)
            b["instructions"] = out
        for sub in b.get("blocks", []) or []:
            fix_block(sub)

    for fn in j["functions"]:
        for blk in fn["blocks"]:
            fix_block(blk)
    return json.dumps(j).encode()


def _install_waitfix(nc):
    orig = nc.to_json_bytes
    nc.to_json_bytes = lambda: _split_multiwaits(orig())


CFG = {
    "xp": 2, "qk": 2, "vp": 2, "ptp": 3, "osb": 2, "recp": 2, "yp": 2,
    "ps_qo": 2, "ps_vy": 2, "ps_s": 1, "ps_b": 2,
}
MASK_MODE = "dve"  # "gpsimd" | "dve" | "pe"
ABLATE = set()  # timing experiments: {"norm", "mask", "exp", "final", "qkcopy", "omm", "smm"}


def _build_nc(repeat=1):
    nc = bass.Bass(trn_type="TRN2", debug=False)
    _install_waitfix(nc)
    f32, f32r, bf16 = DT.float32, DT.float32r, DT.bfloat16

    xT_d = nc.dram_tensor("xT", [BS, IN_DIM, L], f32r, kind="ExternalInput")
    wq_d = nc.dram_tensor("wq", [IN_DIM, D_MODEL], f32r, kind="ExternalInput")
    wk_d = nc.dram_tensor("wk", [IN_DIM, D_MODEL], f32r, kind="ExternalInput")
    wv_d = nc.dram_tensor("wv", [IN_DIM, D_MODEL], f32r, kind="ExternalInput")
    wf_d = nc.dram_tensor("wf", [4, IN_DIM, OUT_DIM], f32r, kind="ExternalInput")
    bq_d = nc.dram_tensor("bq", [IN_DIM, 4], f32, kind="ExternalInput")
    bk_d = nc.dram_tensor("bk", [IN_DIM, 4], f32, kind="ExternalInput")
    bv_d = nc.dram_tensor("bv", [1, D_MODEL], f32r, kind="ExternalInput")
    bf_d = nc.dram_tensor("bf", [1, OUT_DIM], f32r, kind="ExternalInput")
    mask_d = nc.dram_tensor("mask01", [2, 88, L], bf16, kind="ExternalInput")
    madd_d = nc.dram_tensor("madd", [2, 88, L], bf16, kind="ExternalInput")
    i88_d = nc.dram_tensor("i88", [88, 88], bf16, kind="ExternalInput")
    ones_d = nc.dram_tensor("ones", [1, IN_DIM], f32r, kind="ExternalInput")
    y_d = nc.dram_tensor("y", [BS, L, OUT_DIM], f32, kind="ExternalOutput")

    Copy = mybir.ActivationFunctionType.Copy
    Ident = mybir.ActivationFunctionType.Identity
    Exp = mybir.ActivationFunctionType.Exp
    Relu = mybir.ActivationFunctionType.Relu

    with tile.TileContext(nc) as tc:
        with (
            tc.tile_pool(name="consts", bufs=1) as cp,
            tc.tile_pool(name="xp", bufs=CFG["xp"]) as xp,
            tc.tile_pool(name="qk", bufs=CFG["qk"]) as qkp,
            tc.tile_pool(name="vp", bufs=CFG["vp"]) as vp,
            tc.tile_pool(name="ptp", bufs=CFG["ptp"]) as ptp,
            tc.tile_pool(name="osb", bufs=CFG["osb"]) as osbp,
            tc.tile_pool(name="recp", bufs=CFG["recp"]) as recp,
            tc.tile_pool(name="yp", bufs=CFG["yp"]) as yp,
            tc.tile_pool(name="ps_qo", bufs=CFG["ps_qo"], space="PSUM") as pp_qo,
            tc.tile_pool(name="ps_vy", bufs=CFG["ps_vy"], space="PSUM") as pp_vy,
            tc.tile_pool(name="ps_s", bufs=CFG["ps_s"], space="PSUM") as pp_s,
            tc.tile_pool(name="ps_b", bufs=CFG["ps_b"], space="PSUM") as pp_b,
        ):
            wq = cp.tile([IN_DIM, D_MODEL], f32r)
            nc.sync.dma_start(wq[:], wq_d.ap()[:])
            wk = cp.tile([IN_DIM, D_MODEL], f32r)
            nc.sync.dma_start(wk[:], wk_d.ap()[:])
            wv = cp.tile([IN_DIM, D_MODEL], f32r)
            nc.sync.dma_start(wv[:], wv_d.ap()[:])
            wf = cp.tile([IN_DIM, 4 * OUT_DIM], f32r)
            for c in range(4):
                nc.sync.dma_start(wf[:, 512 * c:512 * (c + 1)], wf_d.ap()[c])
            bq = cp.tile([IN_DIM, 4], f32)
            nc.sync.dma_start(bq[:], bq_d.ap()[:])
            bk = cp.tile([IN_DIM, 4], f32)
            nc.sync.dma_start(bk[:], bk_d.ap()[:])
            bv = cp.tile([1, D_MODEL], f32r)
            nc.sync.dma_start(bv[:], bv_d.ap()[:])
            bf_t = cp.tile([1, OUT_DIM], f32r)
            nc.sync.dma_start(bf_t[:], bf_d.ap()[:])
            if MASK_MODE == "pe":
                madd = cp.tile([88, 2 * L], bf16)
                for kc in range(2):
                    nc.sync.dma_start(madd[:, L * kc:L * (kc + 1)],
                                      madd_d.ap()[kc])
                i88 = cp.tile([88, 88], bf16)
                nc.sync.dma_start(i88[:], i88_d.ap()[:])
            else:
                mask01 = cp.tile([88, 2 * L], bf16)
                for kc in range(2):
                    nc.sync.dma_start(mask01[:, L * kc:L * (kc + 1)],
                                      mask_d.ap()[kc])
            ones = cp.tile([1, IN_DIM], f32r)
            nc.sync.dma_start(ones[:], ones_d.ap()[:])

            for sp_i in range((BS // 2) * repeat):
                s0 = (2 * sp_i) % BS
                # two samples share the projection stage: fp32r matmuls at
                # N=352 amortize the serial 4-byte weight load much better
                # than two N=176 ones.
                xt = xp.tile([IN_DIM, 2 * L], f32r)
                for sl in range(2):
                    nc.sync.dma_start(xt[:, L * sl:L * (sl + 1)],
                                      xT_d.ap()[s0 + sl])

                # q^T / k^T projections: psum [128, 352] per 128-chunk of
                # d_model; bias added during PSUM->SBUF copy on ScalarE.
                # Layout: chunk c at cols 352c, sample sl at +176*sl.
                qt = qkp.tile([IN_DIM, 8 * L], f32r, name="qt")
                kt = qkp.tile([IN_DIM, 8 * L], f32r, name="kt")
                for w_t, b_t, dst in ((wq, bq, qt), (wk, bk, kt)):
                    for c in range(4):
                        pq = pp_qo.tile([IN_DIM, 2 * L], f32, name="pq",
                                        tag="qo")
                        nc.tensor.matmul(
                            pq[:], w_t[:, 128 * c:128 * (c + 1)], xt[:],
                            start=True, stop=True,
                        )
                        if "qkcopy" not in ABLATE:
                            nc.scalar.activation(
                                dst[:, 2 * L * c:2 * L * (c + 1)], pq[:],
                                Ident, bias=b_t[:, c:c + 1],
                            )

                for sl in range(2):
                    s = s0 + sl
                    # v: natural layout, keys on partitions, ones-augmented
                    va = []
                    for rc in range(2):
                        pv = pp_vy.tile([88, D_MODEL], f32, name="pv",
                                        tag="vy")
                        nc.tensor.matmul(
                            pv[:],
                            xt[:, L * sl + 88 * rc:L * sl + 88 * (rc + 1)],
                            wv[:], start=True, stop=False,
                        )
                        nc.tensor.matmul(
                            pv[:], ones[:, 0:88], bv[:], start=False,
                            stop=True,
                        )
                        vt = vp.tile([88, 8 * 65], bf16, name=f"va{rc}")
                        vv = vt[:].rearrange("p (h w) -> p h w", w=65)
                        pvv = pv[:].rearrange("p (h w) -> p h w", w=64)
                        nc.scalar.activation(vv[:, :, 0:64], pvv[:], Relu)
                        nc.gpsimd.memset(vv[:, :, 64:65], 1.0)
                        va.append(vt)

                    osb = osbp.tile([IN_DIM, 4 * L], f32r, name="osb")

                    def emit_s(hp):
                        # S^T matmuls for the head pair interleaved: even head
                        # occupies PE rows 0-63, odd head rows 64-127 -> the
                        # weight loads/matmuls of the two heads overlap in the
                        # array (disjoint row groups).
                        sps = []
                        for kc in range(2):
                            for hs in range(2):
                                hr = 64 * hs
                                if kc == 0 and len(sps) < 2:
                                    sps.append(pp_s.tile([88, 2 * L], f32,
                                                         name=f"sp{hs}"))
                                base = 2 * L * hp + L * sl
                                nc.tensor.matmul(
                                    sps[hs][:, L * kc:L * (kc + 1)],
                                    kt[hr:hr + 64,
                                       base + 88 * kc:base + 88 * (kc + 1)],
                                    qt[hr:hr + 64, base:base + L],
                                    start=True, stop=(MASK_MODE != "pe"),
                                )
                                if MASK_MODE == "pe":
                                    nc.tensor.matmul(
                                        sps[hs][:, L * kc:L * (kc + 1)],
                                        i88[:], madd[:, L * kc:L * (kc + 1)],
                                        start=False, stop=True,
                                    )
                        return sps

                    def emit_chain(hp, sps):
                        for hs in range(2):
                            h, hr = 2 * hp + hs, 64 * hs
                            pt = ptp.tile([88, 2 * L], bf16, name=f"pt{hs}")
                            if "exp" not in ABLATE:
                                nc.scalar.activation(pt[:], sps[hs][:], Exp)
                            if "mask" not in ABLATE:
                                if MASK_MODE == "gpsimd":
                                    nc.gpsimd.tensor_mul(pt[:], pt[:], mask01[:])
                                elif MASK_MODE == "dve":
                                    nc.vector.tensor_mul(pt[:], pt[:], mask01[:])

                            po = pp_qo.tile([65, L], f32, name="po", tag="qo")
                            if "omm" not in ABLATE:
                                for kc in range(2):
                                    nc.tensor.matmul(
                                        po[:], va[kc][:, 65 * h:65 * h + 65],
                                        pt[:, L * kc:L * (kc + 1)],
                                        start=(kc == 0), stop=(kc == 1),
                                    )
                            if "norm" not in ABLATE:
                                rec = recp.tile([1, L], f32r, name="rec")
                                with nc.allow_low_precision(reason="f32r recip"):
                                    nc.vector.reciprocal(rec[:], po[64:65, :])
                                pb = pp_b.tile([64, L], f32, name="pb")
                                nc.tensor.matmul(pb[:], ones[:, 0:64], rec[:],
                                                 start=True, stop=True)
                            dst = osb[hr:hr + 64, L * hp:L * (hp + 1)]
                            if hs == 0:
                                nc.scalar.activation(dst, po[0:64, :], Copy)
                            else:
                                nc.vector.tensor_copy(dst, po[0:64, :])
                            if "norm" not in ABLATE:
                                nc.vector.tensor_mul(dst, dst, pb[:])

                    # software pipeline: keep a ready S^T pair queued ahead of
                    # the softmax/normalize chain so PE never head-of-line
                    # blocks on ScalarE/VectorE.
                    prev = None
                    for hp in range(4):
                        sps = emit_s(hp)
                        if prev is not None:
                            emit_chain(hp - 1, prev)
                        prev = sps
                    emit_chain(3, prev)

                    for rc in range(2):
                        py = pp_vy.tile([88, OUT_DIM], f32, name="py", tag="vy")
                        for c in range(4):
                            nc.tensor.matmul(
                                py[:],
                                osb[:, L * c + 88 * rc:L * c + 88 * (rc + 1)],
                                wf[:, 512 * c:512 * (c + 1)],
                                start=(c == 0), stop=False,
                            )
                        nc.tensor.matmul(py[:], ones[:, 0:88], bf_t[:],
                                         start=False, stop=True)
                        ysb = yp.tile([88, OUT_DIM], f32, name="ysb")
                        if rc == 0:
                            nc.vector.tensor_copy(ysb[:], py[:])
                        else:
                            nc.scalar.activation(ysb[:], py[:], Copy)
                        nc.sync.dma_start(
                            y_d.ap()[s, 88 * rc:88 * (rc + 1), :], ysb[:],
                        )
    return nc


def _make_consts():
    frame = np.arange(L) // FRAME
    same_frame = frame[:, None] == frame[None, :]
    mask01 = np.where(same_frame & ~np.eye(L, dtype=bool), np.float32(0.0),
                      np.float32(1.0))
    madd = np.where(same_frame & ~np.eye(L, dtype=bool), np.float32(NEG),
                    np.float32(0.0))
    import ml_dtypes
    return {
        "mask01": np.stack([mask01[0:88], mask01[88:176]]).astype(
            ml_dtypes.bfloat16),
        "madd": np.stack([madd[0:88], madd[88:176]]).astype(ml_dtypes.bfloat16),
        "i88": np.eye(88, dtype=np.float32).astype(ml_dtypes.bfloat16),
        "ones": np.ones((1, IN_DIM), dtype=np.float32),
    }


_NC_CACHE = None


def kernel(x, Wq, bq, Wk, bk, Wv, bv, Wf, bf):
    global _NC_CACHE
    x = np.asarray(x, dtype=np.float32)
    if _NC_CACHE is None:
        _NC_CACHE = _build_nc()
    nc = _NC_CACHE

    consts = _make_consts()
    xT = np.ascontiguousarray(x.transpose(0, 2, 1))  # [B, 128, 176]
    base = {
        "wq": np.asarray(Wq, np.float32) * SCALE,  # fold 1/sqrt(H_DIM) into q
        "wk": np.asarray(Wk, np.float32),
        "wv": np.asarray(Wv, np.float32),
        "wf": np.ascontiguousarray(
            np.asarray(Wf, np.float32).reshape(4, IN_DIM, OUT_DIM)),
        "bq": np.ascontiguousarray(
            (np.asarray(bq, np.float32) * SCALE).reshape(4, IN_DIM).T),
        "bk": np.ascontiguousarray(np.asarray(bk, np.float32).reshape(4, IN_DIM).T),
        "bv": np.asarray(bv, np.float32).reshape(1, D_MODEL),
        "bf": np.asarray(bf, np.float32).reshape(1, OUT_DIM),
        **consts,
    }
    in_maps = [
        {**base, "xT": np.ascontiguousarray(xT[BS * c:BS * (c + 1)])}
        for c in range(N_CORES)
    ]
    global _last_in_maps
    _last_in_maps = in_maps
    res = run_bass_kernel_spmd(nc, in_maps, core_ids=list(range(N_CORES)))
    return np.concatenate([r["y"] for r in res.results], axis=0)


_last_in_maps = None



# revision 44
# speedup vs baseline: 13.3581x; 13.3581x over previous
"""Trainium2 Bass kernel for nn_MultiHeadedAttention_64665027608991.

Sparse (per-frame-masked) multi-head attention over B=512 samples, L=176
(8 frames x 22 joints), 8 heads x 64 dims, fp32 I/O.

Strategy: pure data parallel over batch (64 samples per NeuronCore x 8).
All matmuls run in bf16 (fp32 PSUM accumulate): rel-err budget is 2e-2 and
bf16 keeps us ~5e-3 while making the N=176 score matmuls 4x faster than
fp32r (which drops to 1/4 rate below N=256 on TRN2).

Per sample pair, fully unrolled inside an optional For_i repeat loop:
  - x^T (host-pre-transposed, bf16) -> q^T/k^T via bf16 matmuls at N=352;
    biases folded into the PSUM->SBUF copy on ScalarE.
  - v natural layout with bias via K=1 ones matmul, ReLU on ScalarE into a
    ones-augmented bf16 tile (65 cols per head; col 64 = 1.0 for row sums).
  - scores S^T[k,q] per head-pair interleaved on PE rows 0-63/64-127;
    exp on ScalarE (no max subtraction: |scores| <= ~3) -> bf16 P^T;
    mask multiply split across DVE and GpSimd.
  - O^T accumulated per head pair into one [65, 352] PSUM tile; row 64 is
    the softmax denominator; one DVE reciprocal per pair, GpSimd
    partition_broadcast, then fused DVE multiply (PSUM x SBUF -> bf16 osb).
  - final projection at M=128 (3 query-chunks per sample pair) + bias, y
    written as bf16 and widened to fp32 on the host.
"""

import sys

sys.path.insert(0, "/opt/trn_rl_repo")

import json

import numpy as np

import concourse.bass as bass
import concourse.tile as tile
from concourse import mybir
from concourse.alu_op_type import AluOpType
from concourse.bass import broadcast_tensor_aps
from concourse.bass_utils import run_bass_kernel_spmd

DT = mybir.dt

N_CORES = 8
B = 512
BS = B // N_CORES  # 64 samples per core
L = 176
FRAME = 22
NFRAME = 8
IN_DIM = 128
D_MODEL = 512
H_NUM = 8
H_DIM = 64
OUT_DIM = 512
SCALE = 1.0 / np.sqrt(np.float32(H_DIM))


# ---------------------------------------------------------------------------
# Workaround: the walrus build in this container rejects instructions with
# more than one sync-wait. Split extras onto single-wait EventSemaphore
# carriers on the same engine.
def _split_multiwaits(bir_json_bytes: bytes) -> bytes:
    j = json.loads(bir_json_bytes)
    n = [0]

    def fix_block(b):
        insts = b.get("instructions")
        if insts:
            out = []
            for inst in insts:
                si = inst.get("sync_info")
                waits = (si or {}).get("on_wait") or []
                if len(waits) > 1:
                    for w in waits[:-1]:
                        n[0] += 1
                        out.append({
                            "name": f"waitfix_{n[0]}",
                            "opcode": "EventSemaphore",
                            "engine": inst.get("engine"),
                            "ins": [],
                            "outs": [],
                            "sync_info": {"on_update": [], "on_wait": [w]},
                        })
                    si["on_wait"] = [waits[-1]]
                out.append(inst)
            b["instructions"] = out
        for sub in b.get("blocks", []) or []:
            fix_block(sub)

    for fn in j["functions"]:
        for blk in fn["blocks"]:
            fix_block(blk)
    return json.dumps(j).encode()


def _install_waitfix(nc):
    orig = nc.to_json_bytes
    nc.to_json_bytes = lambda: _split_multiwaits(orig())


def _build_nc(repeat=1):
    nc = bass.Bass(trn_type="TRN2", debug=False)
    _install_waitfix(nc)
    f32, f32r, bf16 = DT.float32, DT.float32r, DT.bfloat16

    xT_d = nc.dram_tensor("xT", [BS, IN_DIM, L], bf16, kind="ExternalInput")
    wq_d = nc.dram_tensor("wq", [IN_DIM, D_MODEL], bf16, kind="ExternalInput")
    wk_d = nc.dram_tensor("wk", [IN_DIM, D_MODEL], bf16, kind="ExternalInput")
    wv_d = nc.dram_tensor("wv", [IN_DIM, D_MODEL], bf16, kind="ExternalInput")
    wf_d = nc.dram_tensor("wf", [4, IN_DIM, OUT_DIM], bf16, kind="ExternalInput")
    bq_d = nc.dram_tensor("bq", [IN_DIM, 4], f32, kind="ExternalInput")
    bk_d = nc.dram_tensor("bk", [IN_DIM, 4], f32, kind="ExternalInput")
    bv_d = nc.dram_tensor("bv", [1, D_MODEL], bf16, kind="ExternalInput")
    bf_d = nc.dram_tensor("bf", [1, OUT_DIM], bf16, kind="ExternalInput")
    bqr_d = nc.dram_tensor("bqr", [1, D_MODEL], bf16, kind="ExternalInput")
    bkr_d = nc.dram_tensor("bkr", [1, D_MODEL], bf16, kind="ExternalInput")
    mask_d = nc.dram_tensor("mask01", [2, 88, L], bf16, kind="ExternalInput")
    y_d = nc.dram_tensor("y", [BS, L, OUT_DIM], bf16, kind="ExternalOutput")

    Ident = mybir.ActivationFunctionType.Identity
    Exp = mybir.ActivationFunctionType.Exp
    Relu = mybir.ActivationFunctionType.Relu

    with tile.TileContext(nc) as tc:
        with (
            tc.tile_pool(name="consts", bufs=1) as cp,
            tc.tile_pool(name="xp", bufs=2) as xp,
            tc.tile_pool(name="qk", bufs=2) as qkp,
            tc.tile_pool(name="vp", bufs=2) as vp,
            tc.tile_pool(name="ptp", bufs=3) as ptp,
            tc.tile_pool(name="osb", bufs=2) as osbp,
            tc.tile_pool(name="recp", bufs=2) as recp,
            tc.tile_pool(name="yp", bufs=2) as yp,
            tc.tile_pool(name="ps_q", bufs=2, space="PSUM") as pp_q,
            tc.tile_pool(name="ps_x", bufs=2, space="PSUM") as pp_x,
            tc.tile_pool(name="ps_s", bufs=2, space="PSUM") as pp_s,
        ):
            wq = cp.tile([IN_DIM, D_MODEL], bf16)
            nc.sync.dma_start(wq[:], wq_d.ap()[:])
            wk = cp.tile([IN_DIM, D_MODEL], bf16)
            nc.sync.dma_start(wk[:], wk_d.ap()[:])
            wv = cp.tile([IN_DIM, D_MODEL], bf16)
            nc.sync.dma_start(wv[:], wv_d.ap()[:])
            wf = cp.tile([IN_DIM, 4 * OUT_DIM], bf16)
            for c in range(4):
                nc.sync.dma_start(wf[:, 512 * c:512 * (c + 1)], wf_d.ap()[c])
            bq = cp.tile([IN_DIM, 4], f32)
            nc.sync.dma_start(bq[:], bq_d.ap()[:])
            bk = cp.tile([IN_DIM, 4], f32)
            nc.sync.dma_start(bk[:], bk_d.ap()[:])
            bv = cp.tile([1, D_MODEL], bf16)
            nc.sync.dma_start(bv[:], bv_d.ap()[:])
            bf_t = cp.tile([1, OUT_DIM], bf16)
            nc.sync.dma_start(bf_t[:], bf_d.ap()[:])
            bqr = cp.tile([1, D_MODEL], bf16)
            nc.sync.dma_start(bqr[:], bqr_d.ap()[:])
            bkr = cp.tile([1, D_MODEL], bf16)
            nc.sync.dma_start(bkr[:], bkr_d.ap()[:])
            mask01 = cp.tile([88, 2 * L], bf16)
            for kc in range(2):
                nc.sync.dma_start(mask01[:, L * kc:L * (kc + 1)],
                                  mask_d.ap()[kc])
            ones = cp.tile([1, OUT_DIM], bf16)
            nc.gpsimd.memset(ones[:], 1.0)

            FINAL_QCHUNKS = ((0, 128), (128, 128), (256, 96))

            def make_final(s0, osb):
                # final projection for a pair at M=128/128/96 over the 352
                # queries; bias added during the PSUM->bf16 copies on DVE
                # (bfb tile) except chunk 1 which copies on ScalarE with a
                # K=1 ones bias matmul. Split into per-chunk closures so the
                # matmuls interleave with the next pair's score chains.
                ysb = yp.tile([IN_DIM, 3 * OUT_DIM], bf16, name="ysb")

                def chunk(j):
                    q0, qn = FINAL_QCHUNKS[j]
                    py = pp_x.tile([IN_DIM, OUT_DIM], f32, name="py", tag="x")
                    for c in range(4):
                        nc.tensor.matmul(
                            py[0:qn, :],
                            osb[:, 2 * L * c + q0:2 * L * c + q0 + qn],
                            wf[:, 512 * c:512 * (c + 1)],
                            start=(c == 0), stop=False,
                        )
                    nc.tensor.matmul(py[0:qn, :], ones[:, 0:qn], bf_t[:],
                                     start=False, stop=True)
                    dsl = ysb[0:qn, OUT_DIM * j:OUT_DIM * (j + 1)]
                    if j == 1:
                        nc.scalar.activation(dsl, py[0:qn, :], Ident)
                    else:
                        nc.vector.tensor_copy(dsl, py[0:qn, :])

                def flush():
                    yflat = y_d.ap()[s0:s0 + 2].rearrange("s q o -> (s q) o")
                    for j, (q0, qn) in enumerate(FINAL_QCHUNKS):
                        nc.sync.dma_start(
                            yflat[q0:q0 + qn, :],
                            ysb[0:qn, OUT_DIM * j:OUT_DIM * (j + 1)])

                return chunk, flush

            def body():
                pending_final = [None]

                for sp_i in range(BS // 2):
                    s0 = 2 * sp_i
                    # two samples share the projection stage (N=352 matmuls)
                    xt = xp.tile([IN_DIM, 2 * L], bf16)
                    for sl in range(2):
                        nc.sync.dma_start(xt[:, L * sl:L * (sl + 1)],
                                          xT_d.ap()[s0 + sl])

                    # q^T / k^T projections: psum [128, 352] per 128-chunk of
                    # d_model; bias added during the PSUM->SBUF copy (chunks
                    # 0-1 on ScalarE, 2-3 on DVE with a free-dim-broadcast
                    # bias column). Layout: chunk c at cols 352c, sample sl
                    # at +176*sl.
                    qt = qkp.tile([IN_DIM, 8 * L], bf16, name="qt")
                    kt = qkp.tile([IN_DIM, 8 * L], bf16, name="kt")
                    for w_t, b_t, br_t, dst in ((wq, bq, bqr, qt),
                                                (wk, bk, bkr, kt)):
                        for c in range(4):
                            pq = pp_q.tile([IN_DIM, 2 * L], f32, name="pq",
                                           tag="q")
                            if c < 3:
                                nc.tensor.matmul(
                                    pq[:], w_t[:, 128 * c:128 * (c + 1)],
                                    xt[:], start=True, stop=True,
                                )
                                nc.scalar.activation(
                                    dst[:, 2 * L * c:2 * L * (c + 1)], pq[:],
                                    Ident, bias=b_t[:, c:c + 1])
                            else:
                                # chunk 3: bias via K=1 ones matmul in PSUM,
                                # plain copy on DVE (keeps ScalarE for exp).
                                nc.tensor.matmul(
                                    pq[:], w_t[:, 128 * c:128 * (c + 1)],
                                    xt[:], start=True, stop=False,
                                )
                                nc.tensor.matmul(
                                    pq[:], br_t[:, 128 * c:128 * (c + 1)],
                                    ones[:, 0:2 * L], start=False, stop=True,
                                )
                                nc.vector.tensor_copy(
                                    dst[:, 2 * L * c:2 * L * (c + 1)], pq[:])

                    # O^T for the whole pair: col block 352c + 176*sl + q
                    osb = osbp.tile([IN_DIM, 8 * L], bf16, name="osb")

                    va = {}

                    def emit_v(sl):
                        # v: natural layout, keys on partitions, ones-augmented
                        va[sl] = []
                        for rc in range(2):
                            pv = pp_x.tile([88, D_MODEL], f32, name="pv",
                                           tag="x")
                            nc.tensor.matmul(
                                pv[:],
                                xt[:, L * sl + 88 * rc:L * sl + 88 * (rc + 1)],
                                wv[:], start=True, stop=False,
                            )
                            nc.tensor.matmul(
                                pv[:], ones[:, 0:88], bv[:], start=False,
                                stop=True,
                            )
                            vt = vp.tile([88, 8 * 65], bf16,
                                         name=f"va{sl}_{rc}")
                            vv = vt[:].rearrange("p (h w) -> p h w", w=65)
                            pvv = pv[:].rearrange("p (h w) -> p h w", w=64)
                            if rc == 0:
                                nc.scalar.activation(vv[:, :, 0:64], pvv[:],
                                                     Relu)
                            else:
                                nc.vector.tensor_scalar_max(vv[:, :, 0:64],
                                                            pvv[:], 0.0)
                            nc.gpsimd.memset(vv[:, :, 64:65], 1.0)
                            va[sl].append(vt)

                    def emit_s(sl, hp):
                        # S^T matmuls for the head pair interleaved: even
                        # head occupies PE rows 0-63, odd head rows 64-127
                        # -> weight loads overlap matmuls (disjoint rows).
                        sps = []
                        for kc in range(2):
                            for hs in range(2):
                                hr = 64 * hs
                                if kc == 0 and len(sps) < 2:
                                    sps.append(pp_s.tile(
                                        [88, 2 * L], f32, name=f"sp{hs}"))
                                base = 2 * L * hp + L * sl
                                nc.tensor.matmul(
                                    sps[hs][:, L * kc:L * (kc + 1)],
                                    kt[hr:hr + 64,
                                       base + 88 * kc:base + 88 * (kc + 1)],
                                    qt[hr:hr + 64, base:base + L],
                                    start=True, stop=True,
                                )
                        return sps

                    def emit_chain(sl, hp, sps):
                        # exp -> mask -> O^T accumulation for the head pair
                        # into one [65, 352] PSUM tile (head hs at cols
                        # 176*hs); row 64 = softmax denominators.
                        po = pp_x.tile([65, 2 * L], f32, name="po", tag="x")
                        pts = []
                        for hs in range(2):
                            h = 2 * hp + hs
                            pt = ptp.tile([88, 2 * L], bf16, name=f"pt{hs}")
                            nc.scalar.activation(pt[:], sps[hs][:], Exp)
                            nc.gpsimd.tensor_mul(pt[:], pt[:], mask01[:])
                            pts.append(pt)
                        for hs in range(2):
                            h = 2 * hp + hs
                            for kc in range(2):
                                nc.tensor.matmul(
                                    po[:, L * hs:L * (hs + 1)],
                                    va[sl][kc][:, 65 * h:65 * h + 65],
                                    pts[hs][:, L * kc:L * (kc + 1)],
                                    start=(kc == 0), stop=(kc == 1),
                                )
                        # normalize: reciprocal of the denominator row (DVE,
                        # PSUM->SBUF), splat across 64 partitions via a K=1
                        # ones matmul (GPSIMD cannot touch PSUM and no engine
                        # allows partition-stride-0 APs), then plain copies
                        # (ScalarE/DVE) + in-place DVE multiply.
                        rec = recp.tile([1, 2 * L], bf16, name="rec")
                        with nc.allow_low_precision(reason="bf16 recip"):
                            nc.vector.reciprocal(rec[:], po[64:65, :])
                        pb = pp_q.tile([64, 2 * L], f32, name="pb", tag="q")
                        nc.tensor.matmul(pb[:], ones[:, 0:64], rec[:],
                                         start=True, stop=True)
                        for hs in range(2):
                            dst = osb[64 * hs:64 * hs + 64,
                                      2 * L * hp + L * sl:
                                      2 * L * hp + L * sl + L]
                            src = po[0:64, L * hs:L * (hs + 1)]
                            if hs == 0:
                                nc.scalar.activation(dst, src, Ident)
                            else:
                                nc.vector.tensor_copy(dst, src)
                            nc.vector.tensor_mul(
                                dst, dst, pb[:, L * hs:L * (hs + 1)])

                    # Interleaved schedule: the two samples' score ("s") and
                    # softmax-chain ("c") stages alternate, with the previous
                    # pair's final-projection chunks ("f") as PE filler, so
                    # each chain's exp->mask latency hides behind ~2us of
                    # independent PE work.
                    fin = pending_final[0]
                    sched = (
                        ("v", 0), ("s", 0, 0), ("v", 1), ("f", 0),
                        ("s", 1, 0), ("c", 0, 0), ("f", 1),
                        ("s", 0, 1), ("c", 1, 0), ("f", 2),
                        ("s", 1, 1), ("c", 0, 1),
                        ("s", 0, 2), ("c", 1, 1),
                        ("s", 1, 2), ("c", 0, 2),
                        ("s", 0, 3), ("c", 1, 2),
                        ("s", 1, 3), ("c", 0, 3), ("c", 1, 3),
                    )
                    live = {}
                    for op in sched:
                        if op[0] == "v":
                            emit_v(op[1])
                        elif op[0] == "s":
                            live[op[1:]] = emit_s(op[1], op[2])
                        elif op[0] == "c":
                            emit_chain(op[1], op[2], live.pop(op[1:]))
                        elif fin is not None:
                            fin[0](op[1])
                            if op[1] == 2:
                                fin[1]()
                    pending_final[0] = make_final(s0, osb)

                if pending_final[0] is not None:
                    fin = pending_final[0]
                    for j in range(3):
                        fin[0](j)
                    fin[1]()
                    pending_final[0] = None

            if repeat == 1:
                body()
            else:
                with tc.For_i(0, repeat):
                    body()
    return nc


def _make_consts():
    frame = np.arange(L) // FRAME
    same_frame = frame[:, None] == frame[None, :]
    mask01 = np.where(same_frame & ~np.eye(L, dtype=bool), np.float32(0.0),
                      np.float32(1.0))
    import ml_dtypes
    return {
        "mask01": np.stack([mask01[0:88], mask01[88:176]]).astype(
            ml_dtypes.bfloat16),
    }


_NC_CACHE = None


def _host_prep(x, Wq, bq, Wk, bk, Wv, bv, Wf, bf):
    import ml_dtypes
    bfloat16 = ml_dtypes.bfloat16
    x = np.asarray(x, dtype=np.float32)
    consts = _make_consts()
    xT = np.ascontiguousarray(x.transpose(0, 2, 1)).astype(bfloat16)
    base = {
        "wq": (np.asarray(Wq, np.float32) * SCALE).astype(bfloat16),
        "wk": np.asarray(Wk, np.float32).astype(bfloat16),
        "wv": np.asarray(Wv, np.float32).astype(bfloat16),
        "wf": np.ascontiguousarray(
            np.asarray(Wf, np.float32).reshape(4, IN_DIM, OUT_DIM)).astype(
                bfloat16),
        "bq": np.ascontiguousarray(
            (np.asarray(bq, np.float32) * SCALE).reshape(4, IN_DIM).T),
        "bk": np.ascontiguousarray(
            np.asarray(bk, np.float32).reshape(4, IN_DIM).T),
        "bv": np.asarray(bv, np.float32).reshape(1, D_MODEL).astype(bfloat16),
        "bf": np.asarray(bf, np.float32).reshape(1, OUT_DIM).astype(bfloat16),
        "bqr": (np.asarray(bq, np.float32) * SCALE).reshape(1, D_MODEL).astype(
            bfloat16),
        "bkr": np.asarray(bk, np.float32).reshape(1, D_MODEL).astype(bfloat16),
        **consts,
    }
    return [
        {**base, "xT": np.ascontiguousarray(xT[BS * c:BS * (c + 1)])}
        for c in range(N_CORES)
    ]


def kernel(x, Wq, bq, Wk, bk, Wv, bv, Wf, bf):
    global _NC_CACHE
    if _NC_CACHE is None:
        _NC_CACHE = _build_nc()
    nc = _NC_CACHE

    in_maps = _host_prep(x, Wq, bq, Wk, bk, Wv, bv, Wf, bf)
    global _last_in_maps
    _last_in_maps = in_maps
    res = run_bass_kernel_spmd(nc, in_maps, core_ids=list(range(N_CORES)))
    return np.concatenate(
        [np.asarray(r["y"]).astype(np.float32) for r in res.results], axis=0)


_last_in_maps = None


# revision 55
# speedup vs baseline: 14.7622x; 1.1051x over previous
"""Trainium2 Bass kernel for nn_MultiHeadedAttention_64665027608991.

Sparse (per-frame-masked) multi-head attention over B=512 samples, L=176
(8 frames x 22 joints), 8 heads x 64 dims, fp32 I/O.

Strategy: pure data parallel over batch (64 samples per NeuronCore x 8).
All matmuls run in bf16 (fp32 PSUM accumulate): rel-err budget is 2e-2 and
bf16 keeps us ~6e-3 while making the N=176 score matmuls 4x faster than
fp32r (which drops to 1/4 rate below N=256 on TRN2). x and y travel the
wire as bf16 (halves per-exec host I/O).

Per sample pair, fully unrolled inside an optional For_i repeat loop (the
hardware loop lets test.py measure an honest repeat-slope exec time):
  - x^T (host-pre-transposed, bf16) -> q^T/k^T via bf16 matmuls at N=352;
    biases folded into the PSUM->SBUF copies (ScalarE activation bias /
    K=1 ones-row matmul for the DVE-copied chunk).
  - v natural layout with bias via K=1 ones matmul, ReLU into a
    ones-augmented bf16 tile (65 cols per head; col 64 = 1.0 for row sums).
  - scores S^T[k,q] per head-pair interleaved on PE rows 0-63/64-127; exp
    on ScalarE (no max subtraction: |scores| <= ~3) and GpSimd mask
    multiply emitted eagerly in the same block.
  - O^T per (head, pair) into one [65, 352] PSUM tile covering BOTH
    samples; row 64 is the softmax denominator; DVE reciprocal, then a
    deferred normalize block (K=1 ones-matmul splat + copy + DVE multiply)
    scheduled ~3 slots later so its latency hides under score matmuls.
  - final projection at M=128/128/96 over the pair's 352 queries,
    software-pipelined one pair behind and interleaved into the next
    pair's schedule as PE filler; y written as bf16, widened on the host.

Engine/PSUM notes: GPSIMD cannot touch PSUM on this toolchain, there is
no partition-broadcast anywhere (K=1 ones matmuls serve as splats), and
PSUM pools are bank-quantized (8 banks exactly: proj 2, pv/py/pb 2,
scores 2, O 2).
"""

import sys

sys.path.insert(0, "/opt/trn_rl_repo")

import json

import numpy as np

import concourse.bass as bass
import concourse.tile as tile
from concourse import mybir
from concourse.alu_op_type import AluOpType
from concourse.bass import broadcast_tensor_aps
from concourse.bass_utils import run_bass_kernel_spmd

DT = mybir.dt

N_CORES = 8
B = 512
BS = B // N_CORES  # 64 samples per core
L = 176
FRAME = 22
NFRAME = 8
IN_DIM = 128
D_MODEL = 512
H_NUM = 8
H_DIM = 64
OUT_DIM = 512
SCALE = 1.0 / np.sqrt(np.float32(H_DIM))

# timing-ablation switches (results become wrong; timing-only experiments)
ABLATE = set()

# engine assignment knobs, tuned empirically on HW (DVE measured ~1.6x the
# cost model on this part; ScalarE has slack)
CFG = {
    "qk3": "act",      # 4th qk-chunk PSUM->SBUF copy: "act" | "dve"
    "relu1": "act",    # 2nd v-relu: "act" | "dve"
    "ocopy1": "act",   # odd-head osb copy: "act" | "dve"
    "ysb": "aad",      # ysb copy engines for chunks j=0,1,2: "a"=ACT "d"=DVE
    "recip": "dve",    # softmax reciprocal engine ("dve" only; Pool has no
                       # divide ALU op and no PSUM access)
}


# ---------------------------------------------------------------------------
# Workaround: the walrus build in this container rejects instructions with
# more than one sync-wait. Split extras onto single-wait EventSemaphore
# carriers on the same engine.
def _split_multiwaits(bir_json_bytes: bytes) -> bytes:
    j = json.loads(bir_json_bytes)
    n = [0]

    def fix_block(b):
        insts = b.get("instructions")
        if insts:
            out = []
            for inst in insts:
                si = inst.get("sync_info")
                waits = (si or {}).get("on_wait") or []
                if len(waits) > 1:
                    for w in waits[:-1]:
                        n[0] += 1
                        out.append({
                            "name": f"waitfix_{n[0]}",
                            "opcode": "EventSemaphore",
                            "engine": inst.get("engine"),
                            "ins": [],
                            "outs": [],
                            "sync_info": {"on_update": [], "on_wait": [w]},
                        })
                    si["on_wait"] = [waits[-1]]
                out.append(inst)
            b["instructions"] = out
        for sub in b.get("blocks", []) or []:
            fix_block(sub)

    for fn in j["functions"]:
        for blk in fn["blocks"]:
            fix_block(blk)
    return json.dumps(j).encode()


def _install_waitfix(nc):
    orig = nc.to_json_bytes
    nc.to_json_bytes = lambda: _split_multiwaits(orig())


def _build_nc(repeat=1):
    nc = bass.Bass(trn_type="TRN2", debug=False)
    _install_waitfix(nc)
    f32, f32r, bf16 = DT.float32, DT.float32r, DT.bfloat16

    xT_d = nc.dram_tensor("xT", [BS, IN_DIM, L], bf16, kind="ExternalInput")
    wq_d = nc.dram_tensor("wq", [IN_DIM, D_MODEL], bf16, kind="ExternalInput")
    wk_d = nc.dram_tensor("wk", [IN_DIM, D_MODEL], bf16, kind="ExternalInput")
    wv_d = nc.dram_tensor("wv", [IN_DIM, D_MODEL], bf16, kind="ExternalInput")
    wf_d = nc.dram_tensor("wf", [4, IN_DIM, OUT_DIM], bf16, kind="ExternalInput")
    bq_d = nc.dram_tensor("bq", [IN_DIM, 4], f32, kind="ExternalInput")
    bk_d = nc.dram_tensor("bk", [IN_DIM, 4], f32, kind="ExternalInput")
    bv_d = nc.dram_tensor("bv", [1, D_MODEL], bf16, kind="ExternalInput")
    bf_d = nc.dram_tensor("bf", [1, OUT_DIM], bf16, kind="ExternalInput")
    bqr_d = nc.dram_tensor("bqr", [1, D_MODEL], bf16, kind="ExternalInput")
    bkr_d = nc.dram_tensor("bkr", [1, D_MODEL], bf16, kind="ExternalInput")
    mask_d = nc.dram_tensor("mask01", [2, 88, L], bf16, kind="ExternalInput")
    y_d = nc.dram_tensor("y", [BS, L, OUT_DIM], bf16, kind="ExternalOutput")

    Ident = mybir.ActivationFunctionType.Identity
    Exp = mybir.ActivationFunctionType.Exp
    Relu = mybir.ActivationFunctionType.Relu

    with tile.TileContext(nc) as tc:
        with (
            tc.tile_pool(name="consts", bufs=1) as cp,
            tc.tile_pool(name="xp", bufs=2) as xp,
            tc.tile_pool(name="qk", bufs=2) as qkp,
            tc.tile_pool(name="vp", bufs=2) as vp,
            tc.tile_pool(name="ptp", bufs=3) as ptp,
            tc.tile_pool(name="osb", bufs=2) as osbp,
            tc.tile_pool(name="recp", bufs=2) as recp,
            tc.tile_pool(name="yp", bufs=2) as yp,
            tc.tile_pool(name="ps_q", bufs=2, space="PSUM") as pp_q,
            tc.tile_pool(name="ps_x", bufs=2, space="PSUM") as pp_x,
            tc.tile_pool(name="ps_s", bufs=1, space="PSUM") as pp_s,
            tc.tile_pool(name="ps_o", bufs=1, space="PSUM") as pp_o,
        ):
            wq = cp.tile([IN_DIM, D_MODEL], bf16)
            nc.sync.dma_start(wq[:], wq_d.ap()[:])
            wk = cp.tile([IN_DIM, D_MODEL], bf16)
            nc.sync.dma_start(wk[:], wk_d.ap()[:])
            wv = cp.tile([IN_DIM, D_MODEL], bf16)
            nc.sync.dma_start(wv[:], wv_d.ap()[:])
            wf = cp.tile([IN_DIM, 4 * OUT_DIM], bf16)
            for c in range(4):
                nc.sync.dma_start(wf[:, 512 * c:512 * (c + 1)], wf_d.ap()[c])
            bq = cp.tile([IN_DIM, 4], f32)
            nc.sync.dma_start(bq[:], bq_d.ap()[:])
            bk = cp.tile([IN_DIM, 4], f32)
            nc.sync.dma_start(bk[:], bk_d.ap()[:])
            bv = cp.tile([1, D_MODEL], bf16)
            nc.sync.dma_start(bv[:], bv_d.ap()[:])
            bf_t = cp.tile([1, OUT_DIM], bf16)
            nc.sync.dma_start(bf_t[:], bf_d.ap()[:])
            bqr = cp.tile([1, D_MODEL], bf16)
            nc.sync.dma_start(bqr[:], bqr_d.ap()[:])
            bkr = cp.tile([1, D_MODEL], bf16)
            nc.sync.dma_start(bkr[:], bkr_d.ap()[:])
            mask01 = cp.tile([88, 2 * L], bf16)
            for kc in range(2):
                nc.sync.dma_start(mask01[:, L * kc:L * (kc + 1)],
                                  mask_d.ap()[kc])
            ones = cp.tile([1, OUT_DIM], bf16)
            nc.gpsimd.memset(ones[:], 1.0)
            onec = cp.tile([88, 1], bf16)
            nc.gpsimd.memset(onec[:], 1.0)

            FINAL_QCHUNKS = ((0, 128), (128, 128), (256, 96))

            def make_final(s0, osb):
                # final projection for a pair at M=128/128/96 over the 352
                # queries; bias added during the PSUM->bf16 copies on DVE
                # (bfb tile) except chunk 1 which copies on ScalarE with a
                # K=1 ones bias matmul. Split into per-chunk closures so the
                # matmuls interleave with the next pair's score chains.
                ysb = yp.tile([IN_DIM, 3 * OUT_DIM], bf16, name="ysb")

                def chunk(j):
                    q0, qn = FINAL_QCHUNKS[j]
                    py = pp_x.tile([IN_DIM, OUT_DIM], f32, name="py", tag="x")
                    if "fmm" not in ABLATE:
                        for c in range(4):
                            nc.tensor.matmul(
                                py[0:qn, :],
                                osb[:, 2 * L * c + q0:2 * L * c + q0 + qn],
                                wf[:, 512 * c:512 * (c + 1)],
                                start=(c == 0), stop=False,
                            )
                        nc.tensor.matmul(py[0:qn, :], ones[:, 0:qn], bf_t[:],
                                         start=False, stop=True)
                    dsl = ysb[0:qn, OUT_DIM * j:OUT_DIM * (j + 1)]
                    if "ysb" not in ABLATE:
                        if CFG["ysb"][j] == "a":
                            nc.scalar.activation(dsl, py[0:qn, :], Ident)
                        else:
                            nc.vector.tensor_copy(dsl, py[0:qn, :])

                def flush():
                    if "ydma" in ABLATE:
                        return
                    yflat = y_d.ap()[s0:s0 + 2].rearrange("s q o -> (s q) o")
                    for j, (q0, qn) in enumerate(FINAL_QCHUNKS):
                        nc.sync.dma_start(
                            yflat[q0:q0 + qn, :],
                            ysb[0:qn, OUT_DIM * j:OUT_DIM * (j + 1)])

                return chunk, flush

            def body():
                pending_final = [None]

                for sp_i in range(BS // 2):
                    s0 = 2 * sp_i
                    # two samples share the projection stage (N=352 matmuls)
                    xt = xp.tile([IN_DIM, 2 * L], bf16)
                    for sl in range(2):
                        nc.sync.dma_start(xt[:, L * sl:L * (sl + 1)],
                                          xT_d.ap()[s0 + sl])

                    # q^T / k^T projections: psum [128, 352] per 128-chunk of
                    # d_model; bias added during the PSUM->SBUF copy (chunks
                    # 0-1 on ScalarE, 2-3 on DVE with a free-dim-broadcast
                    # bias column). Layout: chunk c at cols 352c, sample sl
                    # at +176*sl.
                    qt = qkp.tile([IN_DIM, 8 * L], bf16, name="qt")
                    kt = qkp.tile([IN_DIM, 8 * L], bf16, name="kt")
                    for w_t, b_t, br_t, dst in ((wq, bq, bqr, qt),
                                                (wk, bk, bkr, kt)):
                        for c in range(4):
                            pq = pp_q.tile([IN_DIM, 2 * L], f32, name="pq",
                                           tag="q")
                            if c < 3:
                                nc.tensor.matmul(
                                    pq[:], w_t[:, 128 * c:128 * (c + 1)],
                                    xt[:], start=True, stop=True,
                                )
                                if "qkcopy" not in ABLATE:
                                    nc.scalar.activation(
                                        dst[:, 2 * L * c:2 * L * (c + 1)],
                                        pq[:], Ident, bias=b_t[:, c:c + 1])
                            else:
                                # chunk 3: bias via K=1 ones matmul in PSUM,
                                # plain copy on DVE (keeps ScalarE for exp).
                                nc.tensor.matmul(
                                    pq[:], w_t[:, 128 * c:128 * (c + 1)],
                                    xt[:], start=True, stop=False,
                                )
                                nc.tensor.matmul(
                                    pq[:], br_t[:, 128 * c:128 * (c + 1)],
                                    ones[:, 0:2 * L], start=False, stop=True,
                                )
                                if "qkcopy" not in ABLATE:
                                    if CFG["qk3"] == "act":
                                        nc.scalar.activation(
                                            dst[:, 2 * L * c:2 * L * (c + 1)],
                                            pq[:], Ident)
                                    else:
                                        nc.vector.tensor_copy(
                                            dst[:, 2 * L * c:2 * L * (c + 1)],
                                            pq[:])

                    # O^T for the whole pair: col block 352c + 176*sl + q
                    osb = osbp.tile([IN_DIM, 8 * L], bf16, name="osb")

                    va = {}

                    def emit_v(sl):
                        # v: natural layout, keys on partitions, ones-augmented
                        va[sl] = []
                        for rc in range(2):
                            pv = pp_x.tile([88, D_MODEL], f32, name="pv",
                                           tag="x")
                            nc.tensor.matmul(
                                pv[:],
                                xt[:, L * sl + 88 * rc:L * sl + 88 * (rc + 1)],
                                wv[:], start=True, stop=False,
                            )
                            nc.tensor.matmul(
                                pv[:], ones[:, 0:88], bv[:], start=False,
                                stop=True,
                            )
                            vt = vp.tile([88, 8 * 65], bf16,
                                         name=f"va{sl}_{rc}")
                            vv = vt[:].rearrange("p (h w) -> p h w", w=65)
                            pvv = pv[:].rearrange("p (h w) -> p h w", w=64)
                            if "relu" not in ABLATE:
                                if rc == 0 or CFG["relu1"] == "act":
                                    nc.scalar.activation(vv[:, :, 0:64],
                                                         pvv[:], Relu)
                                else:
                                    nc.vector.tensor_scalar_max(
                                        vv[:, :, 0:64], pvv[:], 0.0)
                            nc.gpsimd.memset(vv[:, :, 64:65], 1.0)
                            va[sl].append(vt)

                    def emit_s(sl, hp):
                        # S^T matmuls for the head pair interleaved: even
                        # head occupies PE rows 0-63, odd head rows 64-127
                        # -> weight loads overlap matmuls (disjoint rows).
                        # exp + mask follow immediately so P^T is ready well
                        # before the O matmuls consume it.
                        sps = []
                        for kc in range(2):
                            for hs in range(2):
                                hr = 64 * hs
                                if kc == 0 and len(sps) < 2:
                                    sps.append(pp_s.tile(
                                        [88, 2 * L], f32, name=f"sp{hs}"))
                                base = 2 * L * hp + L * sl
                                if "smm" not in ABLATE:
                                    nc.tensor.matmul(
                                        sps[hs][:, L * kc:L * (kc + 1)],
                                        kt[hr:hr + 64,
                                           base + 88 * kc:base + 88 * (kc + 1)],
                                        qt[hr:hr + 64, base:base + L],
                                        start=True, stop=True,
                                    )
                        pts = []
                        for hs in range(2):
                            pt = ptp.tile([88, 2 * L], bf16,
                                          name=f"pt{sl}_{hs}")
                            if "exp" not in ABLATE:
                                nc.scalar.activation(pt[:], sps[hs][:], Exp)
                            if "mask" not in ABLATE:
                                nc.gpsimd.tensor_mul(pt[:], pt[:], mask01[:])
                            pts.append(pt)
                        return pts

                    def emit_chain_a(hp, pts01):
                        # O^T for BOTH samples of the pair into one [65, 352]
                        # PSUM tile per head (cols 176*sl + q); row 64 is the
                        # softmax denominator; reciprocal right after.
                        pts0, pts1 = pts01
                        pos, recs = [], []
                        for hs in range(2):
                            h = 2 * hp + hs
                            po = pp_o.tile([65, 2 * L], f32,
                                           name=f"po{hs}")
                            rec = recp.tile([1, 2 * L], bf16,
                                            name=f"rec{hs}")
                            for sl, pts in ((0, pts0), (1, pts1)):
                                for kc in range(2):
                                    if "omm" not in ABLATE:
                                        nc.tensor.matmul(
                                            po[:, L * sl:L * (sl + 1)],
                                            va[sl][kc][:, 65 * h:65 * h + 65],
                                            pts[hs][:, L * kc:L * (kc + 1)],
                                            start=(kc == 0), stop=(kc == 1),
                                        )
                            if "recip" not in ABLATE:
                                with nc.allow_low_precision(
                                        reason="bf16 recip"):
                                    nc.vector.reciprocal(rec[:], po[64:65, :])
                            pos.append(po)
                            recs.append(rec)
                        return pos, recs

                    def emit_chain_b(hp, state):
                        # deferred normalize: splat 1/den via K=1 ones matmul,
                        # copy O^T out of PSUM ([64, 352] covering both
                        # samples), multiply in place.
                        pos, recs = state
                        for hs in range(2):
                            po, rec = pos[hs], recs[hs]
                            pb = pp_x.tile([64, 2 * L], f32, name="pb",
                                           tag="x")
                            if "recip" not in ABLATE:
                                nc.tensor.matmul(pb[:], ones[:, 0:64], rec[:],
                                                 start=True, stop=True)
                            dst = osb[64 * hs:64 * hs + 64,
                                      2 * L * hp:2 * L * (hp + 1)]
                            src = po[0:64, :]
                            if "ocopy" not in ABLATE:
                                if hs == 0 or CFG["ocopy1"] == "act":
                                    nc.scalar.activation(dst, src, Ident)
                                else:
                                    nc.vector.tensor_copy(dst, src)
                            if "omult" not in ABLATE:
                                nc.vector.tensor_mul(dst, dst, pb[:])

                    # Interleaved schedule: the two samples' score ("s"),
                    # chain-A ("a": exp/mask/den/O/recip) and deferred
                    # chain-B ("b": splat/copy/mult) stages alternate, with
                    # the previous pair's final-projection chunks ("f") as PE
                    # filler, so every cross-engine latency hides behind
                    # independent PE work.
                    fin = pending_final[0]
                    sched = (
                        ("v", 0), ("s", 0, 0), ("v", 1), ("s", 1, 0),
                        ("f", 0), ("s", 0, 1), ("a", 0), ("s", 1, 1),
                        ("f", 1), ("b", 0), ("s", 0, 2), ("a", 1),
                        ("s", 1, 2), ("f", 2), ("b", 1), ("s", 0, 3),
                        ("a", 2), ("s", 1, 3), ("b", 2), ("a", 3),
                        ("b", 3),
                    )
                    live = {}
                    for op in sched:
                        if op[0] == "v":
                            emit_v(op[1])
                        elif op[0] == "s":
                            live[op[1:]] = emit_s(op[1], op[2])
                        elif op[0] == "a":
                            hp = op[1]
                            live[("po", hp)] = emit_chain_a(
                                hp, (live.pop((0, hp)), live.pop((1, hp))))
                        elif op[0] == "b":
                            emit_chain_b(op[1], live.pop(("po", op[1])))
                        elif fin is not None:
                            fin[0](op[1])
                            if op[1] == 2:
                                fin[1]()
                    pending_final[0] = make_final(s0, osb)

                if pending_final[0] is not None:
                    fin = pending_final[0]
                    for j in range(3):
                        fin[0](j)
                    fin[1]()
                    pending_final[0] = None

            if repeat == 1:
                body()
            else:
                with tc.For_i(0, repeat):
                    body()
    return nc


def _make_consts():
    frame = np.arange(L) // FRAME
    same_frame = frame[:, None] == frame[None, :]
    mask01 = np.where(same_frame & ~np.eye(L, dtype=bool), np.float32(0.0),
                      np.float32(1.0))
    import ml_dtypes
    return {
        "mask01": np.stack([mask01[0:88], mask01[88:176]]).astype(
            ml_dtypes.bfloat16),
    }


_NC_CACHE = None


def _host_prep(x, Wq, bq, Wk, bk, Wv, bv, Wf, bf):
    import ml_dtypes
    bfloat16 = ml_dtypes.bfloat16
    x = np.asarray(x, dtype=np.float32)
    consts = _make_consts()
    xT = np.ascontiguousarray(x.transpose(0, 2, 1)).astype(bfloat16)
    base = {
        "wq": (np.asarray(Wq, np.float32) * SCALE).astype(bfloat16),
        "wk": np.asarray(Wk, np.float32).astype(bfloat16),
        "wv": np.asarray(Wv, np.float32).astype(bfloat16),
        "wf": np.ascontiguousarray(
            np.asarray(Wf, np.float32).reshape(4, IN_DIM, OUT_DIM)).astype(
                bfloat16),
        "bq": np.ascontiguousarray(
            (np.asarray(bq, np.float32) * SCALE).reshape(4, IN_DIM).T),
        "bk": np.ascontiguousarray(
            np.asarray(bk, np.float32).reshape(4, IN_DIM).T),
        "bv": np.asarray(bv, np.float32).reshape(1, D_MODEL).astype(bfloat16),
        "bf": np.asarray(bf, np.float32).reshape(1, OUT_DIM).astype(bfloat16),
        "bqr": (np.asarray(bq, np.float32) * SCALE).reshape(1, D_MODEL).astype(
            bfloat16),
        "bkr": np.asarray(bk, np.float32).reshape(1, D_MODEL).astype(bfloat16),
        **consts,
    }
    return [
        {**base, "xT": np.ascontiguousarray(xT[BS * c:BS * (c + 1)])}
        for c in range(N_CORES)
    ]


def kernel(x, Wq, bq, Wk, bk, Wv, bv, Wf, bf):
    global _NC_CACHE
    if _NC_CACHE is None:
        _NC_CACHE = _build_nc()
    nc = _NC_CACHE

    in_maps = _host_prep(x, Wq, bq, Wk, bk, Wv, bv, Wf, bf)
    global _last_in_maps
    _last_in_maps = in_maps
    res = run_bass_kernel_spmd(nc, in_maps, core_ids=list(range(N_CORES)))
    return np.concatenate(
        [np.asarray(r["y"]).astype(np.float32) for r in res.results], axis=0)


_last_in_maps = None
